# revision 26
# baseline (speedup 1.0000x reference)
"""Trainium2 Bass kernel for rank-1-projection attention.

Computation (reference, fp32):
    q = x_q @ WQ            [512,512,256]@[256] -> [512,512]
    k = x_k @ WK
    v = x_v @ WV
    y = softmax(q @ k, axis=-1) @ v     -> [512,512]

Strategy: data-parallel over the leading N axis (64 rows/core x 8
cores).  The host packs each core's three x slabs (d-major, fp16) into
one contiguous [48, 128, 4096] stream tensor in consumption order
(k, v, q; per tensor 8 row-ranges x 2 d-halves); the 1 MB stream DMAs
alternate across the two HWDGE rings (1 MB granularity keeps
cross-core skew at the collective barriers small), while every small
latency-critical DMA rides the SWDGE (gpsimd) ring so it never
interacts with the stream's HWDGE FIFOs.  Projections run on the
tensor engine with the x chunk as the stationary operand ([128 d, 128
rows] lhsT x [128, 1] W column -> one fp32 PSUM column).  k and v are
projected first; each pack (psum -> row-major fp16) + AllGather is
issued under tc.high_priority(), and the NEXT projection's W columns
are re-materialized by a scalar-engine copy placed right after the
pack's psum->SBUF copy on the scalar FIFO -- a real dependency that
stops the scheduler from slipping the pack behind the next tensor's
matmul stream.  Both gathers and the single-DMA k/v readbacks hide
under the q stream.  The attention tail is split into two 32-row
chunks (q ranges partition the output rows): chunk 0's
qk/softmax/transpose chain hides under the second half of the q
stream, so only chunk 1's chain + the s@v matmul remain after the
last input byte.  fp16 everywhere except fp32 PSUM accumulation and
the softmax; the fp16 output is widened to fp32 on the host.
"""

import numpy as np

import concourse.bass as bass
import concourse.mybir as mybir
import concourse.tile as tile
from concourse import bacc
from concourse.bass_utils import run_bass_kernel_spmd
from concourse.masks import make_identity

N = 512          # attention size (rows/cols)
D = 256          # projection dim
CORES = 8
NL = N // CORES  # 64 leading rows per core
R = NL * N       # 32768 projection rows per tensor per core
RNG = 4096       # rows per DMA tile ([128, RNG] fp16 = 1 MB)
NRG = R // RNG   # 8 ranges per tensor
CPT = RNG // 128  # 32 chunks of 128 rows per tile

F32 = mybir.dt.float32
F16 = mybir.dt.float16

_CACHE = {}


def _build():
    if "nc" in _CACHE:
        return _CACHE["nc"]

    nc = bacc.Bacc(
        "TRN2", target_bir_lowering=False, debug=False, num_devices=CORES
    )

    # stream tensor: tile t = (tensor, range, half), each [128, RNG]
    # fully contiguous in HBM (tiles in consumption order k, v, q)
    xs = nc.dram_tensor(
        "xs", [3 * NRG * 2, 128, RNG], F16, kind="ExternalInput"
    )
    wall = nc.dram_tensor("wall", [128, 6], F16, kind="ExternalInput")
    yout = nc.dram_tensor("yout", [NL, N], F16, kind="ExternalOutput")

    with tile.TileContext(nc) as tc:
        with (
            tc.tile_pool(name="consts", bufs=1) as consts,
            tc.tile_pool(name="xs", bufs=6) as xs_pool,
            tc.tile_pool(name="psum", bufs=1, space="PSUM") as psum_pool,
            tc.tile_pool(name="dram", bufs=1, space="DRAM") as dram_pool,
        ):
            w_t = consts.tile([128, 6], F16)
            nc.gpsimd.dma_start(w_t[:], wall[:])
            ident = consts.tile([128, 128], F32)
            make_identity(nc, ident[:])

            rings = [nc.sync, nc.scalar, nc.gpsimd]

            # fp32 psum accumulators, [b%128, (b//128)*64 + i] layout:
            # ps[p, bb*64 + a] = proj value of slab row a*512 + bb*128 + p
            # (2-slot rotation: k -> slot0, v -> slot1, q -> slot0 again)
            tcount = [0]

            def project(widx, w2, rgs=None, dest=None):
                if dest is None:
                    dest = psum_pool.tile(
                        [128, 4 * NL], F32, tag="ps", bufs=2, name=f"ps{widx}"
                    )
                for rg in rgs if rgs is not None else range(NRG):
                    tiles = []
                    for h in (0, 1):
                        xt = xs_pool.tile([128, RNG], F16, tag="xt", name="xt")
                        # alternate hwdge rings to hide per-DMA latency gaps
                        rings[h].dma_start(xt[:], xs[tcount[0]])
                        tcount[0] += 1
                        tiles.append(xt)
                    for j in range(CPT):
                        # slab rows rg*RNG + j*128 ... +128:
                        # a = rg*(RNG//512) + j//4, b-block bb = j%4
                        # ->  psum column bb*64 + a
                        col = (j % 4) * NL + rg * (RNG // N) + j // 4
                        for h in (0, 1):
                            nc.tensor.matmul(
                                dest[:, col : col + 1],
                                lhsT=tiles[h][:, j * 128 : (j + 1) * 128],
                                rhs=w2[:, h : h + 1],
                                start=(h == 0),
                                stop=(h == 1),
                            )
                return dest

            # Re-tile a projection psum [128, 256] into [a, b] rows (fp16),
            # DMA to the collective input, trigger the AllGather, and (key)
            # re-materialize the NEXT projection's W columns via the scalar
            # FIFO right after the psum->SBUF copy: the next tensor's
            # matmuls depend on it, so the scheduler must emit this pack's
            # transposes before them -- the pack cannot slip behind the
            # next projection's matmul stream.
            def pack_rows(src_ps, dst, dst_off, next_widx=None):
                w_next = None
                with tc.high_priority():
                    sbt = consts.tile([128, 4 * NL], F32, name=f"sbt{dst_off}")
                    nc.scalar.activation(
                        sbt[:], src_ps[:], mybir.ActivationFunctionType.Copy
                    )
                    if next_widx is not None:
                        w_next = consts.tile(
                            [128, 2], F16, name=f"w{next_widx}"
                        )
                        nc.scalar.activation(
                            w_next[:],
                            w_t[:, 2 * next_widx : 2 * next_widx + 2],
                            mybir.ActivationFunctionType.Copy,
                        )
                    for bb in range(4):
                        pt = psum_pool.tile(
                            [NL, 128], F32, tag="tp", bufs=2, name="pt"
                        )
                        nc.tensor.transpose(
                            pt[:], sbt[:, bb * NL : (bb + 1) * NL], ident[:]
                        )
                        nc.vector.tensor_copy(
                            out=dst[:, dst_off + bb * 128 : dst_off + (bb + 1) * 128],
                            in_=pt[:],
                        )
                return w_next

            kv_loc = consts.tile([NL, 2 * N], F16)
            cc_in_k = dram_pool.tile([NL, N], F16)
            cc_in_v = dram_pool.tile([NL, N], F16)
            cc_out_k = dram_pool.tile([N, N], F16, addr_space="Shared")
            cc_out_v = dram_pool.tile([N, N], F16, addr_space="Shared")

            def gather(cc_in, cc_out, src_cols):
                with tc.high_priority():
                    nc.gpsimd.dma_start(cc_in[:], kv_loc[:, src_cols])
                    nc.gpsimd.collective_compute(
                        "AllGather",
                        mybir.AluOpType.bypass,
                        replica_groups=[list(range(CORES))],
                        ins=[cc_in[:].opt()],
                        outs=[cc_out[:].opt()],
                    )

            # ---- k then v, each gathered right away so the collective
            # overlaps the remaining x streams ----
            ps_k = project(1, w_t[:, 2:4])
            w_v = pack_rows(ps_k, kv_loc, 0, next_widx=2)
            gather(cc_in_k, cc_out_k, slice(0, N))

            ps_v = project(2, w_v[:])
            w_q = pack_rows(ps_v, kv_loc, N, next_widx=0)
            gather(cc_in_v, cc_out_v, slice(N, 2 * N))

            # gathered k/v rows, one DMA each: column block b holds rows
            # b*128..(b+1)*128, i.e. k_sb(b) = k_t[:, b*N:(b+1)*N]
            # (3-D AP: dst [p, b, j] <- src row (b*128+p), col j)
            k_t = consts.tile([128, 4 * N], F16, name="k_t")
            v_t = consts.tile([128, 4 * N], F16, name="v_t")
            nc.gpsimd.dma_start(
                k_t[:].rearrange("p (b j) -> p b j", b=4),
                cc_out_k[:].rearrange("(b p) j -> p b j", b=4),
            )
            nc.gpsimd.dma_start(
                v_t[:].rearrange("p (b j) -> p b j", b=4),
                cc_out_v[:].rearrange("(b p) j -> p b j", b=4),
            )
            k_sb = [k_t[:, b * N : (b + 1) * N] for b in range(4)]
            v_sb = [v_t[:, b * N : (b + 1) * N] for b in range(4)]

            # ---- q projection + chunked attention tail ----
            # q ranges partition the OUTPUT rows (a = slab_row // 512), so
            # after q's first 4 ranges the attention rows 0..31 are fully
            # computable: chunk 0's qk/softmax/transpose chain hides under
            # the second half of the q stream.  The second-half q matmuls
            # read re-materialized W columns (scalar FIFO, after chunk 0's
            # q_sbT copies) so the scheduler cannot emit them before chunk
    # 0's tail.  PSUM chunk bases 0/32 satisfy the PE quadrant rule.
            q_sbT = consts.tile([128, 4 * NL], F16)
            py = psum_pool.tile([NL, N], F32, tag="mm", name="py")
            neg_mx = consts.tile([NL, 1], F32)
            s_sb = consts.tile([NL, N], F32)
            sumexp = consts.tile([NL, 1], F32)
            rsum = consts.tile([NL, 1], F32)
            st2 = [consts.tile([128, NL], F16, name=f"st{b}") for b in range(4)]
            HC = NL // 2  # 32 rows per chunk

            def q_tail_chunk(ps_q, c, next_w):
                w_next = None
                with tc.high_priority():
                    sl = slice(c * HC, (c + 1) * HC)
                    for bb in range(4):
                        cs = slice(bb * NL + c * HC, bb * NL + (c + 1) * HC)
                        nc.scalar.activation(
                            q_sbT[:, cs], ps_q[:, cs],
                            mybir.ActivationFunctionType.Copy,
                        )
                    if next_w is not None:
                        w_next = consts.tile([128, 2], F16, name="wq2")
                        nc.scalar.activation(
                            w_next[:], w_t[:, 0:2],
                            mybir.ActivationFunctionType.Copy,
                        )
                    for b in range(4):
                        nc.tensor.matmul(
                            py[sl, :],
                            lhsT=q_sbT[:, b * NL + c * HC : b * NL + (c + 1) * HC],
                            rhs=k_sb[b],
                            start=(b == 0), stop=(b == 3),
                        )
                    nc.vector.tensor_reduce(
                        out=neg_mx[sl, :], in_=py[sl, :],
                        axis=mybir.AxisListType.X,
                        op=mybir.AluOpType.max, negate=True,
                    )
                    nc.scalar.activation(
                        s_sb[sl, :], py[sl, :],
                        mybir.ActivationFunctionType.Exp,
                        bias=neg_mx[sl, :], scale=1.0,
                        accum_out=sumexp[sl, :],
                    )
                    nc.vector.reciprocal(rsum[sl, :], sumexp[sl, :])
                    for b in range(4):
                        pt2 = psum_pool.tile(
                            [128, HC], F32, tag="tp2", bufs=2, name="pt2"
                        )
                        nc.tensor.transpose(
                            pt2[:], s_sb[sl, b * 128 : (b + 1) * 128],
                            ident[c * HC : (c + 1) * HC, c * HC : (c + 1) * HC],
                        )
                        nc.vector.tensor_copy(
                            out=st2[b][:, c * HC : (c + 1) * HC], in_=pt2[:]
                        )
                return w_next

            ps_q = project(0, w_q[:], rgs=range(0, NRG // 2))
            w_q2 = q_tail_chunk(ps_q, 0, next_w=True)
            project(0, w_q2[:], rgs=range(NRG // 2, NRG), dest=ps_q)
            q_tail_chunk(ps_q, 1, next_w=None)

            po = psum_pool.tile([NL, N], F32, tag="mm2", name="po")
            for b in range(4):
                nc.tensor.matmul(
                    po[:], lhsT=st2[b][:], rhs=v_sb[b],
                    start=(b == 0), stop=(b == 3),
                )

            out_sb = consts.tile([NL, N], F16)
            nc.vector.tensor_scalar_mul(out_sb[:], po[:], rsum[:])
            nc.sync.dma_start(yout[:], out_sb[:])

    nc.compile()
    _CACHE["nc"] = nc
    return nc


def _make_in_maps(inputs):
    x_q = np.asarray(inputs["x_q"], dtype=np.float32)
    x_k = np.asarray(inputs["x_k"], dtype=np.float32)
    x_v = np.asarray(inputs["x_v"], dtype=np.float32)
    w_all = np.stack(
        [
            np.asarray(inputs["WQ"], dtype=np.float32),
            np.asarray(inputs["WK"], dtype=np.float32),
            np.asarray(inputs["WV"], dtype=np.float32),
        ],
        axis=1,
    ).reshape(2, 128, 3).transpose(1, 2, 0).reshape(128, 6)  # [p, 2*tensor+half]
    w_all = np.ascontiguousarray(w_all).astype(np.float16)
    in_maps = []
    for r in range(CORES):
        sl = slice(r * NL, (r + 1) * NL)
        blocks = []
        for x in (x_k, x_v, x_q):
            xt = x[sl].reshape(R, D).T.astype(np.float16)  # [256, 32768]
            for rg in range(NRG):
                for h in (0, 1):
                    blocks.append(
                        xt[h * 128 : (h + 1) * 128, rg * RNG : (rg + 1) * RNG]
                    )
        in_maps.append(
            {
                "xs": np.ascontiguousarray(np.stack(blocks, axis=0)),
                "wall": w_all,
            }
        )
    return in_maps


def _run(inputs, trace=False):
    nc = _build()
    res = run_bass_kernel_spmd(
        nc, _make_in_maps(inputs), core_ids=list(range(CORES)), trace=trace
    )
    out = np.concatenate(
        [res.results[r]["yout"] for r in range(CORES)], axis=0
    ).astype(np.float32)
    return out, res


def kernel(**inputs):
    out, _ = _run(inputs)
    return out



# revision 27
# speedup vs baseline: 1.0082x; 1.0082x over previous
"""Trainium2 Bass kernel for rank-1-projection attention.

Computation (reference, fp32):
    q = x_q @ WQ            [512,512,256]@[256] -> [512,512]
    k = x_k @ WK
    v = x_v @ WV
    y = softmax(q @ k, axis=-1) @ v     -> [512,512]

Strategy: data-parallel over the leading N axis (64 rows/core x 8
cores).  The host packs each core's three x slabs (d-major, fp16) into
one contiguous [48, 128, 4096] stream tensor in consumption order
(k, v, q; per tensor 8 row-ranges x 2 d-halves); the 1 MB stream DMAs
alternate across the two HWDGE rings (1 MB granularity keeps
cross-core skew at the collective barriers small), while every small
latency-critical DMA rides the SWDGE (gpsimd) ring so it never
interacts with the stream's HWDGE FIFOs.  Projections run on the
tensor engine with the x chunk as the stationary operand ([128 d, 128
rows] lhsT x [128, 1] W column -> one fp32 PSUM column).  k and v are
projected first; each pack (psum -> row-major fp16) + AllGather is
issued under tc.high_priority(), and the NEXT projection's W columns
are re-materialized by a scalar-engine copy placed right after the
pack's psum->SBUF copy on the scalar FIFO -- a real dependency that
stops the scheduler from slipping the pack behind the next tensor's
matmul stream.  Both gathers and the single-DMA k/v readbacks hide
under the q stream.  The attention tail is split into two 32-row
chunks (q ranges partition the output rows): chunk 0's
qk/softmax/transpose chain hides under the second half of the q
stream, so only chunk 1's chain + the s@v matmul remain after the
last input byte.  fp16 everywhere except fp32 PSUM accumulation and
the softmax; the fp16 output is widened to fp32 on the host.
"""

import numpy as np

import concourse.bass as bass
import concourse.mybir as mybir
import concourse.tile as tile
from concourse import bacc
from concourse.bass_utils import run_bass_kernel_spmd
from concourse.masks import make_identity

N = 512          # attention size (rows/cols)
D = 256          # projection dim
CORES = 8
NL = N // CORES  # 64 leading rows per core
R = NL * N       # 32768 projection rows per tensor per core
RNG = 4096       # rows per DMA tile ([128, RNG] fp16 = 1 MB)
NRG = R // RNG   # 8 ranges per tensor
CPT = RNG // 128  # 32 chunks of 128 rows per tile

F32 = mybir.dt.float32
F16 = mybir.dt.float16

_CACHE = {}


def _build():
    if "nc" in _CACHE:
        return _CACHE["nc"]

    nc = bacc.Bacc(
        "TRN2", target_bir_lowering=False, debug=False, num_devices=CORES
    )

    # stream tensor: tile t = (tensor, range, half), each [128, RNG]
    # fully contiguous in HBM (tiles in consumption order k, v, q)
    xs = nc.dram_tensor(
        "xs", [3 * NRG * 2, 128, RNG], F16, kind="ExternalInput"
    )
    wall = nc.dram_tensor("wall", [128, 6], F16, kind="ExternalInput")
    yout = nc.dram_tensor("yout", [NL, N], F16, kind="ExternalOutput")

    with tile.TileContext(nc) as tc:
        with (
            tc.tile_pool(name="consts", bufs=1) as consts,
            tc.tile_pool(name="xs", bufs=6) as xs_pool,
            tc.tile_pool(name="psum", bufs=1, space="PSUM") as psum_pool,
            tc.tile_pool(name="dram", bufs=1, space="DRAM") as dram_pool,
        ):
            w_t = consts.tile([128, 6], F16)
            nc.gpsimd.dma_start(w_t[:], wall[:])
            ident = consts.tile([128, 128], F32)
            make_identity(nc, ident[:])

            rings = [nc.sync, nc.scalar, nc.gpsimd]

            # fp32 psum accumulators, [b%128, (b//128)*64 + i] layout:
            # ps[p, bb*64 + a] = proj value of slab row a*512 + bb*128 + p
            # (2-slot rotation: k -> slot0, v -> slot1, q -> slot0 again)
            tcount = [0]

            def project(widx, w2, rgs=None, dest=None):
                if dest is None:
                    dest = psum_pool.tile(
                        [128, 4 * NL], F32, tag="ps", bufs=2, name=f"ps{widx}"
                    )
                for rg in rgs if rgs is not None else range(NRG):
                    last = widx == 0 and rg == NRG - 1
                    tiles = []
                    for h in (0, 1):
                        xt = xs_pool.tile([128, RNG], F16, tag="xt", name="xt")
                        # alternate hwdge rings to hide per-DMA latency gaps;
                        # the final q tiles stream in halves so their matmuls
                        # start ~1us before the last bytes land
                        if last:
                            for hh in (0, 1):
                                rings[h].dma_start(
                                    xt[:, hh * (RNG // 2) : (hh + 1) * (RNG // 2)],
                                    xs[tcount[0]][:, hh * (RNG // 2) : (hh + 1) * (RNG // 2)],
                                )
                        else:
                            rings[h].dma_start(xt[:], xs[tcount[0]])
                        tcount[0] += 1
                        tiles.append(xt)
                    for j in range(CPT):
                        # slab rows rg*RNG + j*128 ... +128:
                        # a = rg*(RNG//512) + j//4, b-block bb = j%4
                        # ->  psum column bb*64 + a
                        col = (j % 4) * NL + rg * (RNG // N) + j // 4
                        for h in (0, 1):
                            nc.tensor.matmul(
                                dest[:, col : col + 1],
                                lhsT=tiles[h][:, j * 128 : (j + 1) * 128],
                                rhs=w2[:, h : h + 1],
                                start=(h == 0),
                                stop=(h == 1),
                            )
                return dest

            # Re-tile a projection psum [128, 256] into [a, b] rows (fp16),
            # DMA to the collective input, trigger the AllGather, and (key)
            # re-materialize the NEXT projection's W columns via the scalar
            # FIFO right after the psum->SBUF copy: the next tensor's
            # matmuls depend on it, so the scheduler must emit this pack's
            # transposes before them -- the pack cannot slip behind the
            # next projection's matmul stream.
            def pack_rows(src_ps, dst, dst_off, next_widx=None):
                w_next = None
                with tc.high_priority():
                    sbt = consts.tile([128, 4 * NL], F32, name=f"sbt{dst_off}")
                    nc.scalar.activation(
                        sbt[:], src_ps[:], mybir.ActivationFunctionType.Copy
                    )
                    if next_widx is not None:
                        w_next = consts.tile(
                            [128, 2], F16, name=f"w{next_widx}"
                        )
                        nc.scalar.activation(
                            w_next[:],
                            w_t[:, 2 * next_widx : 2 * next_widx + 2],
                            mybir.ActivationFunctionType.Copy,
                        )
                    for bb in range(4):
                        pt = psum_pool.tile(
                            [NL, 128], F32, tag="tp", bufs=2, name="pt"
                        )
                        nc.tensor.transpose(
                            pt[:], sbt[:, bb * NL : (bb + 1) * NL], ident[:]
                        )
                        nc.vector.tensor_copy(
                            out=dst[:, dst_off + bb * 128 : dst_off + (bb + 1) * 128],
                            in_=pt[:],
                        )
                return w_next

            kv_loc = consts.tile([NL, 2 * N], F16)
            cc_in_k = dram_pool.tile([NL, N], F16)
            cc_in_v = dram_pool.tile([NL, N], F16)
            cc_out_k = dram_pool.tile([N, N], F16, addr_space="Shared")
            cc_out_v = dram_pool.tile([N, N], F16, addr_space="Shared")

            def gather(cc_in, cc_out, src_cols):
                with tc.high_priority():
                    nc.gpsimd.dma_start(cc_in[:], kv_loc[:, src_cols])
                    nc.gpsimd.collective_compute(
                        "AllGather",
                        mybir.AluOpType.bypass,
                        replica_groups=[list(range(CORES))],
                        ins=[cc_in[:].opt()],
                        outs=[cc_out[:].opt()],
                    )

            # ---- k then v, each gathered right away so the collective
            # overlaps the remaining x streams ----
            ps_k = project(1, w_t[:, 2:4])
            w_v = pack_rows(ps_k, kv_loc, 0, next_widx=2)
            gather(cc_in_k, cc_out_k, slice(0, N))

            ps_v = project(2, w_v[:])
            w_q = pack_rows(ps_v, kv_loc, N, next_widx=0)
            gather(cc_in_v, cc_out_v, slice(N, 2 * N))

            # gathered k/v rows, one DMA each: column block b holds rows
            # b*128..(b+1)*128, i.e. k_sb(b) = k_t[:, b*N:(b+1)*N]
            # (3-D AP: dst [p, b, j] <- src row (b*128+p), col j)
            k_t = consts.tile([128, 4 * N], F16, name="k_t")
            v_t = consts.tile([128, 4 * N], F16, name="v_t")
            nc.gpsimd.dma_start(
                k_t[:].rearrange("p (b j) -> p b j", b=4),
                cc_out_k[:].rearrange("(b p) j -> p b j", b=4),
            )
            nc.gpsimd.dma_start(
                v_t[:].rearrange("p (b j) -> p b j", b=4),
                cc_out_v[:].rearrange("(b p) j -> p b j", b=4),
            )
            k_sb = [k_t[:, b * N : (b + 1) * N] for b in range(4)]
            v_sb = [v_t[:, b * N : (b + 1) * N] for b in range(4)]

            # ---- q projection + chunked attention tail ----
            # q ranges partition the OUTPUT rows (a = slab_row // 512), so
            # after q's first 4 ranges the attention rows 0..31 are fully
            # computable: chunk 0's qk/softmax/transpose chain hides under
            # the second half of the q stream.  The second-half q matmuls
            # read re-materialized W columns (scalar FIFO, after chunk 0's
            # q_sbT copies) so the scheduler cannot emit them before chunk
    # 0's tail.  PSUM chunk bases 0/32 satisfy the PE quadrant rule.
            q_sbT = consts.tile([128, 4 * NL], F16)
            py = psum_pool.tile([NL, N], F32, tag="mm", name="py")
            st2 = [consts.tile([128, NL], F16, name=f"st{b}") for b in range(4)]
            HC = NL // 2  # 32 rows per chunk
            # per-chunk partition-base-0 tiles: sliced (base-32) operands for
            # chunk 1 cost the ACT engine a slow path (~3.7us MAX->EXP gap)
            neg_mx = [consts.tile([HC, 1], F32, name=f"nm{c}") for c in (0, 1)]
            s_c = [consts.tile([HC, N], F32, name=f"s{c}") for c in (0, 1)]
            sumexp = [consts.tile([HC, 1], F32, name=f"se{c}") for c in (0, 1)]
            rsum = [consts.tile([HC, 1], F32, name=f"rs{c}") for c in (0, 1)]

            def q_tail_chunk(ps_q, c, next_w):
                w_next = None
                with tc.high_priority():
                    sl = slice(c * HC, (c + 1) * HC)
                    for bb in range(4):
                        cs = slice(bb * NL + c * HC, bb * NL + (c + 1) * HC)
                        nc.scalar.activation(
                            q_sbT[:, cs], ps_q[:, cs],
                            mybir.ActivationFunctionType.Copy,
                        )
                    if next_w is not None:
                        w_next = consts.tile([128, 2], F16, name="wq2")
                        nc.scalar.activation(
                            w_next[:], w_t[:, 0:2],
                            mybir.ActivationFunctionType.Copy,
                        )
                    for b in range(4):
                        nc.tensor.matmul(
                            py[sl, :],
                            lhsT=q_sbT[:, b * NL + c * HC : b * NL + (c + 1) * HC],
                            rhs=k_sb[b],
                            start=(b == 0), stop=(b == 3),
                        )
                    nc.vector.tensor_reduce(
                        out=neg_mx[c][:], in_=py[sl, :],
                        axis=mybir.AxisListType.X,
                        op=mybir.AluOpType.max, negate=True,
                    )
                    nc.scalar.activation(
                        s_c[c][:], py[sl, :],
                        mybir.ActivationFunctionType.Exp,
                        bias=neg_mx[c][:], scale=1.0,
                        accum_out=sumexp[c][:],
                    )
                    nc.vector.reciprocal(rsum[c][:], sumexp[c][:])
                    for b in range(4):
                        pt2 = psum_pool.tile(
                            [128, HC], F32, tag="tp2", bufs=2, name="pt2"
                        )
                        nc.tensor.transpose(
                            pt2[:], s_c[c][:, b * 128 : (b + 1) * 128],
                            ident[:HC, :HC],
                        )
                        nc.vector.tensor_copy(
                            out=st2[b][:, c * HC : (c + 1) * HC], in_=pt2[:]
                        )
                return w_next

            ps_q = project(0, w_q[:], rgs=range(0, NRG // 2))
            w_q2 = q_tail_chunk(ps_q, 0, next_w=True)
            project(0, w_q2[:], rgs=range(NRG // 2, NRG), dest=ps_q)
            q_tail_chunk(ps_q, 1, next_w=None)

            po = psum_pool.tile([NL, N], F32, tag="mm2", name="po")
            for b in range(4):
                nc.tensor.matmul(
                    po[:], lhsT=st2[b][:], rhs=v_sb[b],
                    start=(b == 0), stop=(b == 3),
                )

            out_sb = consts.tile([NL, N], F16)
            for c in (0, 1):
                nc.vector.tensor_scalar_mul(
                    out_sb[c * HC : (c + 1) * HC, :],
                    po[c * HC : (c + 1) * HC, :], rsum[c][:],
                )
            nc.sync.dma_start(yout[:], out_sb[:])

    nc.compile()
    _CACHE["nc"] = nc
    return nc


def _make_in_maps(inputs):
    x_q = np.asarray(inputs["x_q"], dtype=np.float32)
    x_k = np.asarray(inputs["x_k"], dtype=np.float32)
    x_v = np.asarray(inputs["x_v"], dtype=np.float32)
    w_all = np.stack(
        [
            np.asarray(inputs["WQ"], dtype=np.float32),
            np.asarray(inputs["WK"], dtype=np.float32),
            np.asarray(inputs["WV"], dtype=np.float32),
        ],
        axis=1,
    ).reshape(2, 128, 3).transpose(1, 2, 0).reshape(128, 6)  # [p, 2*tensor+half]
    w_all = np.ascontiguousarray(w_all).astype(np.float16)
    in_maps = []
    for r in range(CORES):
        sl = slice(r * NL, (r + 1) * NL)
        blocks = []
        for x in (x_k, x_v, x_q):
            xt = x[sl].reshape(R, D).T.astype(np.float16)  # [256, 32768]
            for rg in range(NRG):
                for h in (0, 1):
                    blocks.append(
                        xt[h * 128 : (h + 1) * 128, rg * RNG : (rg + 1) * RNG]
                    )
        in_maps.append(
            {
                "xs": np.ascontiguousarray(np.stack(blocks, axis=0)),
                "wall": w_all,
            }
        )
    return in_maps


def _run(inputs, trace=False):
    nc = _build()
    res = run_bass_kernel_spmd(
        nc, _make_in_maps(inputs), core_ids=list(range(CORES)), trace=trace
    )
    out = np.concatenate(
        [res.results[r]["yout"] for r in range(CORES)], axis=0
    ).astype(np.float32)
    return out, res


def kernel(**inputs):
    out, _ = _run(inputs)
    return out



# revision 28
# speedup vs baseline: 1.0171x; 1.0089x over previous
"""Trainium2 Bass kernel for rank-1-projection attention.

Computation (reference, fp32):
    q = x_q @ WQ            [512,512,256]@[256] -> [512,512]
    k = x_k @ WK
    v = x_v @ WV
    y = softmax(q @ k, axis=-1) @ v     -> [512,512]

Strategy: data-parallel over the leading N axis (64 rows/core x 8
cores).  The host packs each core's three x slabs (d-major, fp16) into
one contiguous [48, 128, 4096] stream tensor in consumption order
(k, v, q; per tensor 8 row-ranges x 2 d-halves); the 1 MB stream DMAs
alternate across the two HWDGE rings (1 MB granularity keeps
cross-core skew at the collective barriers small), while every small
latency-critical DMA rides the SWDGE (gpsimd) ring so it never
interacts with the stream's HWDGE FIFOs.  Projections run on the
tensor engine with the x chunk as the stationary operand ([128 d, 128
rows] lhsT x [128, 1] W column -> one fp32 PSUM column).  k and v are
projected first; each pack (psum -> row-major fp16) + AllGather is
issued under tc.high_priority(), and the NEXT projection's W columns
are re-materialized by a scalar-engine copy placed right after the
pack's psum->SBUF copy on the scalar FIFO -- a real dependency that
stops the scheduler from slipping the pack behind the next tensor's
matmul stream.  Both gathers and the single-DMA k/v readbacks hide
under the q stream.  The attention tail is split into two 32-row
chunks (q ranges partition the output rows): chunk 0's
qk/softmax/transpose chain hides under the second half of the q
stream, so only chunk 1's chain + the s@v matmul remain after the
last input byte.  fp16 everywhere except fp32 PSUM accumulation and
the softmax; the fp16 output is widened to fp32 on the host.
"""

import numpy as np

import concourse.bass as bass
import concourse.mybir as mybir
import concourse.tile as tile
from concourse import bacc
from concourse.bass_utils import run_bass_kernel_spmd
from concourse.masks import make_identity

N = 512          # attention size (rows/cols)
D = 256          # projection dim
CORES = 8
NL = N // CORES  # 64 leading rows per core
R = NL * N       # 32768 projection rows per tensor per core
RNG = 4096       # rows per DMA tile ([128, RNG] fp16 = 1 MB)
NRG = R // RNG   # 8 ranges per tensor
CPT = RNG // 128  # 32 chunks of 128 rows per tile

F32 = mybir.dt.float32
F16 = mybir.dt.float16

_CACHE = {}


def _build():
    if "nc" in _CACHE:
        return _CACHE["nc"]

    nc = bacc.Bacc(
        "TRN2", target_bir_lowering=False, debug=False, num_devices=CORES
    )

    # stream tensor: tile t = (tensor, range, half), each [128, RNG]
    # fully contiguous in HBM (tiles in consumption order k, v, q)
    xs = nc.dram_tensor(
        "xs", [3 * NRG * 2, 128, RNG], F16, kind="ExternalInput"
    )
    wall = nc.dram_tensor("wall", [128, 6], F16, kind="ExternalInput")
    yout = nc.dram_tensor("yout", [NL, N], F16, kind="ExternalOutput")

    with tile.TileContext(nc) as tc:
        with (
            tc.tile_pool(name="consts", bufs=1) as consts,
            tc.tile_pool(name="xs", bufs=6) as xs_pool,
            tc.tile_pool(name="psum", bufs=1, space="PSUM") as psum_pool,
            tc.tile_pool(name="dram", bufs=1, space="DRAM") as dram_pool,
        ):
            w_t = consts.tile([128, 6], F16)
            nc.gpsimd.dma_start(w_t[:], wall[:])
            ident = consts.tile([128, 128], F32)
            make_identity(nc, ident[:])

            rings = [nc.sync, nc.scalar, nc.gpsimd]

            # fp32 psum accumulators, [b%128, (b//128)*64 + i] layout:
            # ps[p, bb*64 + a] = proj value of slab row a*512 + bb*128 + p
            # (2-slot rotation: k -> slot0, v -> slot1, q -> slot0 again)
            tcount = [0]

            def project(widx, w2, rgs=None, dest=None):
                if dest is None:
                    dest = psum_pool.tile(
                        [128, 4 * NL], F32, tag="ps", bufs=2, name=f"ps{widx}"
                    )
                for rg in rgs if rgs is not None else range(NRG):
                    last = widx == 0 and rg == NRG - 1
                    tiles = []
                    for h in (0, 1):
                        xt = xs_pool.tile([128, RNG], F16, tag="xt", name="xt")
                        # alternate hwdge rings to hide per-DMA latency gaps;
                        # the final q tiles stream in halves so their matmuls
                        # start ~1us before the last bytes land
                        if last:
                            for hh in (0, 1):
                                rings[h].dma_start(
                                    xt[:, hh * (RNG // 2) : (hh + 1) * (RNG // 2)],
                                    xs[tcount[0]][:, hh * (RNG // 2) : (hh + 1) * (RNG // 2)],
                                )
                        else:
                            rings[h].dma_start(xt[:], xs[tcount[0]])
                        tcount[0] += 1
                        tiles.append(xt)
                    for j in range(CPT):
                        # slab rows rg*RNG + j*128 ... +128:
                        # a = rg*(RNG//512) + j//4, b-block bb = j%4
                        # ->  psum column bb*64 + a
                        col = (j % 4) * NL + rg * (RNG // N) + j // 4
                        for h in (0, 1):
                            nc.tensor.matmul(
                                dest[:, col : col + 1],
                                lhsT=tiles[h][:, j * 128 : (j + 1) * 128],
                                rhs=w2[:, h : h + 1],
                                start=(h == 0),
                                stop=(h == 1),
                            )
                return dest

            # Re-tile a projection psum [128, 256] into [a, b] rows (fp16),
            # DMA to the collective input, trigger the AllGather, and (key)
            # re-materialize the NEXT projection's W columns via the scalar
            # FIFO right after the psum->SBUF copy: the next tensor's
            # matmuls depend on it, so the scheduler must emit this pack's
            # transposes before them -- the pack cannot slip behind the
            # next projection's matmul stream.
            def pack_rows(src_ps, dst, dst_off, next_widx=None):
                w_next = None
                with tc.high_priority():
                    sbt = consts.tile([128, 4 * NL], F32, name=f"sbt{dst_off}")
                    nc.scalar.activation(
                        sbt[:], src_ps[:], mybir.ActivationFunctionType.Copy
                    )
                    if next_widx is not None:
                        w_next = consts.tile(
                            [128, 2], F16, name=f"w{next_widx}"
                        )
                        nc.scalar.activation(
                            w_next[:],
                            w_t[:, 2 * next_widx : 2 * next_widx + 2],
                            mybir.ActivationFunctionType.Copy,
                        )
                    for bb in range(4):
                        pt = psum_pool.tile(
                            [NL, 128], F32, tag="tp", bufs=2, name="pt"
                        )
                        nc.tensor.transpose(
                            pt[:], sbt[:, bb * NL : (bb + 1) * NL], ident[:]
                        )
                        nc.vector.tensor_copy(
                            out=dst[:, dst_off + bb * 128 : dst_off + (bb + 1) * 128],
                            in_=pt[:],
                        )
                return w_next

            kv_loc = consts.tile([NL, 2 * N], F16)
            cc_in_k = dram_pool.tile([NL, N], F16)
            cc_in_v = dram_pool.tile([NL, N], F16)
            cc_out_k = dram_pool.tile([N, N], F16, addr_space="Shared")
            cc_out_v = dram_pool.tile([N, N], F16, addr_space="Shared")

            def gather(cc_in, cc_out, src_cols):
                with tc.high_priority():
                    nc.gpsimd.dma_start(cc_in[:], kv_loc[:, src_cols])
                    nc.gpsimd.collective_compute(
                        "AllGather",
                        mybir.AluOpType.bypass,
                        replica_groups=[list(range(CORES))],
                        ins=[cc_in[:].opt()],
                        outs=[cc_out[:].opt()],
                    )

            # ---- k then v, each gathered right away so the collective
            # overlaps the remaining x streams ----
            ps_k = project(1, w_t[:, 2:4])
            w_v = pack_rows(ps_k, kv_loc, 0, next_widx=2)
            gather(cc_in_k, cc_out_k, slice(0, N))

            ps_v = project(2, w_v[:])
            w_q = pack_rows(ps_v, kv_loc, N, next_widx=0)
            gather(cc_in_v, cc_out_v, slice(N, 2 * N))

            # gathered k/v rows, one DMA each: column block b holds rows
            # b*128..(b+1)*128, i.e. k_sb(b) = k_t[:, b*N:(b+1)*N]
            # (3-D AP: dst [p, b, j] <- src row (b*128+p), col j)
            k_t = consts.tile([128, 4 * N], F16, name="k_t")
            v_t = consts.tile([128, 4 * N], F16, name="v_t")
            nc.gpsimd.dma_start(
                k_t[:].rearrange("p (b j) -> p b j", b=4),
                cc_out_k[:].rearrange("(b p) j -> p b j", b=4),
            )
            nc.gpsimd.dma_start(
                v_t[:].rearrange("p (b j) -> p b j", b=4),
                cc_out_v[:].rearrange("(b p) j -> p b j", b=4),
            )
            k_sb = [k_t[:, b * N : (b + 1) * N] for b in range(4)]
            v_sb = [v_t[:, b * N : (b + 1) * N] for b in range(4)]

            # ---- q projection + chunked attention tail ----
            # q ranges partition the OUTPUT rows (a = slab_row // 512), so
            # after q's first 4 ranges the attention rows 0..31 are fully
            # computable: chunk 0's qk/softmax/transpose chain hides under
            # the second half of the q stream.  The second-half q matmuls
            # read re-materialized W columns (scalar FIFO, after chunk 0's
            # q_sbT copies) so the scheduler cannot emit them before chunk
    # 0's tail.  PSUM chunk bases 0/32 satisfy the PE quadrant rule.
            q_sbT = consts.tile([128, 4 * NL], F16)
            py = psum_pool.tile([NL, N], F32, tag="mm", name="py")
            st2 = [consts.tile([128, NL], F16, name=f"st{b}") for b in range(4)]
            HC = NL // 2  # 32 rows per chunk
            # per-chunk partition-base-0 tiles: sliced (base-32) operands for
            # chunk 1 cost the ACT engine a slow path (~3.7us MAX->EXP gap)
            neg_mx = [consts.tile([HC, 1], F32, name=f"nm{c}") for c in (0, 1)]
            s_c = [consts.tile([HC, N], F32, name=f"s{c}") for c in (0, 1)]
            sumexp = [consts.tile([HC, 1], F32, name=f"se{c}") for c in (0, 1)]
            rsum = consts.tile([NL, 1], F32)

            def q_tail_chunk(ps_q, c, next_w):
                w_next = None
                with tc.high_priority():
                    sl = slice(c * HC, (c + 1) * HC)
                    for bb in range(4):
                        cs = slice(bb * NL + c * HC, bb * NL + (c + 1) * HC)
                        nc.scalar.activation(
                            q_sbT[:, cs], ps_q[:, cs],
                            mybir.ActivationFunctionType.Copy,
                        )
                    if next_w is not None:
                        w_next = consts.tile([128, 2], F16, name="wq2")
                        nc.scalar.activation(
                            w_next[:], w_t[:, 0:2],
                            mybir.ActivationFunctionType.Copy,
                        )
                    for b in range(4):
                        nc.tensor.matmul(
                            py[sl, :],
                            lhsT=q_sbT[:, b * NL + c * HC : b * NL + (c + 1) * HC],
                            rhs=k_sb[b],
                            start=(b == 0), stop=(b == 3),
                        )
                    nc.vector.tensor_reduce(
                        out=neg_mx[c][:], in_=py[sl, :],
                        axis=mybir.AxisListType.X,
                        op=mybir.AluOpType.max, negate=True,
                    )
                    nc.scalar.activation(
                        s_c[c][:], py[sl, :],
                        mybir.ActivationFunctionType.Exp,
                        bias=neg_mx[c][:], scale=1.0,
                        accum_out=sumexp[c][:],
                    )
                    nc.vector.reciprocal(rsum[sl, :], sumexp[c][:])
                    for b in range(4):
                        pt2 = psum_pool.tile(
                            [128, HC], F32, tag="tp2", bufs=2, name="pt2"
                        )
                        nc.tensor.transpose(
                            pt2[:], s_c[c][:, b * 128 : (b + 1) * 128],
                            ident[:HC, :HC],
                        )
                        nc.vector.tensor_copy(
                            out=st2[b][:, c * HC : (c + 1) * HC], in_=pt2[:]
                        )
                return w_next

            ps_q = project(0, w_q[:], rgs=range(0, NRG // 2))
            w_q2 = q_tail_chunk(ps_q, 0, next_w=True)
            project(0, w_q2[:], rgs=range(NRG // 2, NRG), dest=ps_q)
            q_tail_chunk(ps_q, 1, next_w=None)

            po = psum_pool.tile([NL, N], F32, tag="mm2", name="po")
            for b in range(4):
                nc.tensor.matmul(
                    po[:], lhsT=st2[b][:], rhs=v_sb[b],
                    start=(b == 0), stop=(b == 3),
                )

            out_sb = consts.tile([NL, N], F16)
            nc.vector.tensor_scalar_mul(out_sb[:], po[:], rsum[:])
            nc.sync.dma_start(yout[:], out_sb[:])

    nc.compile()
    _CACHE["nc"] = nc
    return nc


def _make_in_maps(inputs):
    x_q = np.asarray(inputs["x_q"], dtype=np.float32)
    x_k = np.asarray(inputs["x_k"], dtype=np.float32)
    x_v = np.asarray(inputs["x_v"], dtype=np.float32)
    w_all = np.stack(
        [
            np.asarray(inputs["WQ"], dtype=np.float32),
            np.asarray(inputs["WK"], dtype=np.float32),
            np.asarray(inputs["WV"], dtype=np.float32),
        ],
        axis=1,
    ).reshape(2, 128, 3).transpose(1, 2, 0).reshape(128, 6)  # [p, 2*tensor+half]
    w_all = np.ascontiguousarray(w_all).astype(np.float16)
    in_maps = []
    for r in range(CORES):
        sl = slice(r * NL, (r + 1) * NL)
        blocks = []
        for x in (x_k, x_v, x_q):
            xt = x[sl].reshape(R, D).T.astype(np.float16)  # [256, 32768]
            for rg in range(NRG):
                for h in (0, 1):
                    blocks.append(
                        xt[h * 128 : (h + 1) * 128, rg * RNG : (rg + 1) * RNG]
                    )
        in_maps.append(
            {
                "xs": np.ascontiguousarray(np.stack(blocks, axis=0)),
                "wall": w_all,
            }
        )
    return in_maps


def _run(inputs, trace=False):
    nc = _build()
    res = run_bass_kernel_spmd(
        nc, _make_in_maps(inputs), core_ids=list(range(CORES)), trace=trace
    )
    out = np.concatenate(
        [res.results[r]["yout"] for r in range(CORES)], axis=0
    ).astype(np.float32)
    return out, res


def kernel(**inputs):
    out, _ = _run(inputs)
    return out

